# revision 62
# baseline (speedup 1.0000x reference)
"""Causal single-head attention on 8 Trainium2 NeuronCores (batch-parallel), v2.

Problem (nn_Head): x[32,1024,256] f32, Wk/Wq/Wv[64,256] f32.
  q/k/v = x @ W.T ; wei = softmax(causal(q @ k.T / 8)) ; out = wei @ v.

Sharding: B=32 split 4-per-core across 8 cores; weights replicated.

v3 changes vs v2:
  - outputs stored as bf16 in [BPC, 2, P, 4, HS] layout (512B contiguous
    per partition per descriptor, no small-element DMA penalty), upcast
    to f32 on the host: halves the store traffic and shortens the tail.
  - item 0's si0 scores + exp run per t-half with their own PSUM tiles,
    so the scalar engine's exp stream starts as soon as the first x half
    is projected (~2us earlier than waiting for the full A/B casts).
  - diagonal mask-multiplies write SEPARATE ewd tiles instead of masking
    ew in place: Tile tracks whole-tile deps, so in-place masking made
    every PV matmul of a group wait for the pool multiply (~0.7us after
    each exp); with ewd only the one diagonal PV matmul waits.
  - item 0's x pieces ride the sync + scalar queues only (no SWDGE
    desc-gen on the pool engine).
"""

import numpy as np
import ml_dtypes

B, T, C, HS = 32, 1024, 256, 64
NCORES = 8
BPC = B // NCORES  # batch items per core
P = 128            # partitions / row-tile
NT = T // P        # 8 row tiles per item
CO = C // P        # 2 contraction chunks for projections
TCH = 512          # matmul free-dim chunk (one PSUM bank of f32)
N_WARMUP = 7       # dummy matmuls (N=512) to warm the PE clock

_cached = {}


def _build():
    import concourse.tile as tile
    from concourse import bacc, mybir

    bf16 = mybir.dt.bfloat16
    f32 = mybir.dt.float32
    Exp = mybir.ActivationFunctionType.Exp
    Mult = mybir.AluOpType.mult

    nc = bacc.Bacc(
        "TRN2",
        target_bir_lowering=False,
        debug=False,
        num_devices=NCORES,
        monotonic_sem_count=0,
    )

    xT = nc.dram_tensor("xT", [BPC, C, T], bf16, kind="ExternalInput")
    # packed weights, one DMA: per partition p the 768 bf16 columns are
    # [wA(co0)|wA(co1)|wB(co0)|wB(co1)|wV(co0)|wV(co1)|mask]
    wcat = nc.dram_tensor("wcat", [P, 768], bf16, kind="ExternalInput")
    out = nc.dram_tensor("out", [BPC, 2, P, 4, HS], bf16, kind="ExternalOutput")

    # score-group layout: group id -> (si list, psum cols, act window)
    #   groups 0..3 hold a single si in a [P, 1024] tile (2 banks), valid
    #   t from 128*si, exp reads exactly the causal span.
    #   group 4 = {si4, si5} in [P, 1024]: si4 at cols 0:512 (t=512..1024),
    #   si5 at cols 512:1024 (same t window; valid from t=640) -> one exp
    #   of 1024 cols (128 garbage, never read downstream).
    #   group 5 = {si6, si7} in [P, 512] (1 bank): si6 at 0:256
    #   (t=768..1024), si7 at 256:384 (t=896..1024) -> one exp of 384 cols.

    with tile.TileContext(nc) as tc:
        with (
            tc.tile_pool(name="consts", bufs=1) as consts,
            tc.tile_pool(name="xin", bufs=4) as xin,
            tc.tile_pool(name="ab", bufs=4) as abp,
            tc.tile_pool(name="vau", bufs=3) as vaup,
            tc.tile_pool(name="expw", bufs=2) as expwp,
            tc.tile_pool(name="ewd", bufs=2) as ewdp,
            tc.tile_pool(name="outp", bufs=3) as outp,
            tc.tile_pool(name="ps_big", bufs=3, space="PSUM") as ps_big,
            tc.tile_pool(name="ps_sm", bufs=2, space="PSUM") as ps_sm,
        ):
            # ---- input DMAs ---------------------------------------------
            # packed weights blob: one trigger, first thing on the sync
            # queue so it lands before the (larger) x pieces behind it.
            wcat_sb = consts.tile([P, 768], bf16, tag="wcat")
            nc.sync.dma_start(wcat_sb, wcat[:, :])

            def wA_co(co):
                return wcat_sb[:, co * P:(co + 1) * P]

            def wB_co(co):
                return wcat_sb[:, 256 + co * P:256 + (co + 1) * P]

            def wV_co(co):
                return wcat_sb[:, 512 + co * HS:512 + (co + 1) * HS]

            mask_sb = consts.tile([P, P], bf16, tag="mask")
            nc.gpsimd.tensor_copy(mask_sb, wcat_sb[:, 640:768])

            # warmup source first so the gpsimd memset isn't queued behind
            # the DMA trigger instructions
            dummy_src = consts.tile([P, TCH], bf16, tag="dummy")
            nc.gpsimd.memset(dummy_src, 0.0)

            # x loads: co0 on the sync HW queue, co1 on the scalar HW
            # queue.  The DMA engines round-robin across every queued
            # transfer, so item 0 (t-halved, 4 pieces) goes up alone;
            # each later item's DMA is WAW-gated on a tiny copy that
            # depends on the PREVIOUS item's A-cast, which serializes
            # the items on the wire in the order compute needs them.
            xT_tiles = []
            for it in range(BPC):
                t = xin.tile([P, CO, T], bf16, tag="xT", name=f"xT{it}")
                xT_tiles.append(t)
            for h in range(2):
                nc.sync.dma_start(
                    xT_tiles[0][:, 0:1, h * TCH:(h + 1) * TCH],
                    xT[0].rearrange("(co p) t -> p co t", p=P)[
                        :, 0:1, h * TCH:(h + 1) * TCH],
                )
                # both co1 halves on the scalar queue: skipping the
                # gpsimd SWDGE queue avoids its pool-engine desc-gen cost
                nc.scalar.dma_start(
                    xT_tiles[0][:, 1:2, h * TCH:(h + 1) * TCH],
                    xT[0].rearrange("(co p) t -> p co t", p=P)[
                        :, 1:2, h * TCH:(h + 1) * TCH],
                )

            def load_gated(it, gate_ap, co1_eng=None):
                # items 2/3 go sync-only: a late-gated trigger in the
                # scalar engine's stream would block the exp pipeline
                # (in-order engine).  item 1's gate fires before the
                # first exp, so its co1 half may use the scalar queue.
                t = xT_tiles[it]
                r = xT[it].rearrange("(co p) t -> p co t", p=P)
                nc.gpsimd.tensor_copy(t[:, 0, 0:2], gate_ap)
                nc.gpsimd.tensor_copy(t[:, 1, 0:2], gate_ap)
                nc.sync.dma_start(t[:, 0:1, :], r[:, 0:1, :])
                (co1_eng or nc.sync).dma_start(t[:, 1:2, :], r[:, 1:2, :])

            # item 1: gated on item 0's own co1 data having landed.
            # co1 stays on sync too: a gated trigger at the top of the
            # scalar stream would delay the ACT table load behind it.
            load_gated(1, xT_tiles[0][:, 1, T - 2:T])

            # ---- PE warmup ----------------------------------------------
            ps_warm = ps_big.tile([P, 2, TCH], f32, tag="ps", name="warm")
            for w in range(N_WARMUP):
                nc.tensor.matmul(
                    ps_warm[:, w % 2, :],
                    dummy_src[:, 0:P],
                    dummy_src,
                    start=True,
                    stop=True,
                )

            # ---- per-item emitters --------------------------------------
            A_sb = {}
            B_sb = {}
            vaug = {}
            expw = {}   # (item, grp) -> ew tile
            ewd = {}    # (item, si) -> masked diagonal chunk tile
            po = {}     # (item, key) -> psum tile

            def emit_projA(i, ps=None):
                if ps is None:
                    ps = ps_big.tile([P, 2 * TCH], f32, tag="ps",
                                     name=f"pA{i}")
                for h in range(2):
                    for co in range(CO):
                        nc.tensor.matmul(
                            ps[:, h * TCH:(h + 1) * TCH],
                            wA_co(co),
                            xT_tiles[i][:, co, h * TCH:(h + 1) * TCH],
                            start=(co == 0),
                            stop=(co == CO - 1),
                        )
                A_sb[i] = abp.tile([P, T], bf16, tag="A", name=f"A{i}")
                # split cast: first half unblocks the si=0 score matmul
                nc.vector.tensor_copy(A_sb[i][:, 0:TCH], ps[:, 0:TCH])
                if i + 2 < BPC:
                    load_gated(i + 2, A_sb[i][:, 0:2])
                nc.vector.tensor_copy(A_sb[i][:, TCH:T], ps[:, TCH:T])

            def emit_projB(i, ps=None):
                if ps is None:
                    ps = ps_big.tile([P, 2 * TCH], f32, tag="ps",
                                     name=f"pB{i}")
                for h in range(2):
                    for co in range(CO):
                        nc.tensor.matmul(
                            ps[:, h * TCH:(h + 1) * TCH],
                            wB_co(co),
                            xT_tiles[i][:, co, h * TCH:(h + 1) * TCH],
                            start=(co == 0),
                            stop=(co == CO - 1),
                        )
                B_sb[i] = abp.tile([P, T], bf16, tag="B", name=f"B{i}")
                # split cast: the first 256 cols cover si=0/1; for item 0
                # they go on the still-idle scalar engine so the first
                # scores aren't serialized behind the DVE casts
                if i == 0:
                    nc.scalar.copy(B_sb[i][:, 0:P], ps[:, 0:P])
                    nc.scalar.copy(B_sb[i][:, P:2 * P], ps[:, P:2 * P])
                    nc.vector.tensor_copy(B_sb[i][:, 2 * P:T], ps[:, 2 * P:T])
                else:
                    nc.vector.tensor_copy(B_sb[i][:, 0:2 * P], ps[:, 0:2 * P])
                    nc.vector.tensor_copy(B_sb[i][:, 2 * P:T], ps[:, 2 * P:T])

            def emit_projV(i):
                psv = ps_sm.tile([P, NT, HS], f32, tag="sm", name=f"pV{i}")
                for ti in range(NT):
                    for co in range(CO):
                        nc.tensor.matmul(
                            psv[:, ti, :],
                            xT_tiles[i][:, co, ti * P:(ti + 1) * P],
                            wV_co(co),
                            start=(co == 0),
                            stop=(co == CO - 1),
                        )
                v = vaup.tile([P, NT, HS + 1], bf16, tag="vaug", name=f"va{i}")
                nc.gpsimd.memset(v[:, :, HS:HS + 1], 1.0)
                nc.vector.tensor_copy(v[:, :, 0:HS], psv)
                vaug[i] = v

            def emit_diag(i, si, src_ap):
                """Masked diagonal chunk into its own tile: keeps the pool
                multiply OFF the ew tile so PV's off-diagonal reads don't
                wait on it (Tile tracks whole-tile deps)."""
                d = ewdp.tile([P, P], bf16, tag=f"d{si}", name=f"ewd{i}s{si}")
                ewd[(i, si)] = d
                nc.gpsimd.tensor_mul(d, src_ap, mask_sb)

            def emit_scores(i, grp):
                """matmul + exp + (pool) diagonal mask for one score group."""
                if grp < 3:
                    si = grp
                    t_lo = si * P
                    ncols = T - t_lo
                    ps = ps_big.tile([P, 2 * TCH], f32, tag="ps",
                                     name=f"sc{i}g{grp}")
                    for tj in range(2):
                        t0 = max(tj * TCH, t_lo)
                        t1 = (tj + 1) * TCH
                        if t0 >= t1:
                            continue
                        nc.tensor.matmul(
                            ps[:, t0:t1],
                            B_sb[i][0:HS, si * P:(si + 1) * P],
                            A_sb[i][0:HS, t0:t1],
                            start=True,
                            stop=True,
                        )
                    ew = expwp.tile([P, ncols], bf16, tag=f"ew{grp}",
                                    name=f"ew{i}g{grp}")
                    nc.scalar.activation(ew, ps[:, t_lo:T], Exp, scale=0.125)
                    emit_diag(i, si, ew[:, 0:P])
                elif grp == 3:
                    # merged {si3|si6|si7}: blocks 0:5 = si3 (t=384+128b),
                    # 5:7 = si6 (t=768..1024), 7 = si7 (t=896..1024)
                    ps = ps_big.tile([P, 8, P], f32, tag="ps",
                                     name=f"sc{i}g3")
                    nc.tensor.matmul(
                        ps[:, 0:1, :],
                        B_sb[i][0:HS, 3 * P:4 * P],
                        A_sb[i][0:HS, 3 * P:TCH],
                        start=True, stop=True,
                    )
                    nc.tensor.matmul(
                        ps[:, 1:4, :],
                        B_sb[i][0:HS, 3 * P:4 * P],
                        A_sb[i][0:HS, TCH:TCH + 384],
                        start=True, stop=True,
                    )
                    nc.tensor.matmul(
                        ps[:, 4:5, :],
                        B_sb[i][0:HS, 3 * P:4 * P],
                        A_sb[i][0:HS, TCH + 384:T],
                        start=True, stop=True,
                    )
                    nc.tensor.matmul(
                        ps[:, 5:7, :],
                        B_sb[i][0:HS, 6 * P:7 * P],
                        A_sb[i][0:HS, 6 * P:T],
                        start=True, stop=True,
                    )
                    nc.tensor.matmul(
                        ps[:, 7:8, :],
                        B_sb[i][0:HS, 7 * P:T],
                        A_sb[i][0:HS, 7 * P:T],
                        start=True, stop=True,
                    )
                    ew = expwp.tile([P, 8, P], bf16, tag="ew3",
                                    name=f"ew{i}g3")
                    nc.scalar.activation(ew, ps, Exp, scale=0.125)
                    # diagonals: si3 at block 0, si6 at block 5, si7 at 7
                    emit_diag(i, 3, ew[:, 0, :])
                    emit_diag(i, 6, ew[:, 5, :])
                    emit_diag(i, 7, ew[:, 7, :])
                elif grp == 4:
                    # blocks: si4 at 0:4 (t = 512 + 128b), si5 at 4:7
                    # (t = 640 + 128(b-4)); no garbage block.
                    ps = ps_big.tile([P, 7, P], f32, tag="ps",
                                     name=f"sc{i}g4")
                    nc.tensor.matmul(
                        ps[:, 0:4, :],
                        B_sb[i][0:HS, 4 * P:5 * P],
                        A_sb[i][0:HS, TCH:T],
                        start=True,
                        stop=True,
                    )
                    nc.tensor.matmul(
                        ps[:, 4:7, :],
                        B_sb[i][0:HS, 5 * P:6 * P],
                        A_sb[i][0:HS, 5 * P:T],
                        start=True,
                        stop=True,
                    )
                    ew = expwp.tile([P, 7, P], bf16, tag="ew4",
                                    name=f"ew{i}g4")
                    nc.scalar.activation(ew, ps, Exp, scale=0.125)
                    # diagonals: si4 at block 0, si5 at block 4
                    emit_diag(i, 4, ew[:, 0, :])
                    emit_diag(i, 5, ew[:, 4, :])
                expw[(i, grp)] = ew

            def ew_chunk(i, si, ti):
                """128-wide ew column chunk for (si, ti)."""
                if si == ti:
                    return ewd[(i, si)]
                if si < 3:
                    ew = expw[(i, si)]
                    c0 = (ti - si) * P
                    return ew[:, c0:c0 + P]
                if si == 3:
                    return expw[(i, 3)][:, ti - 3, :]
                if si == 4:
                    return expw[(i, 4)][:, ti - 4, :]
                if si == 5:
                    return expw[(i, 4)][:, ti - 1, :]
                if si == 6:
                    return expw[(i, 3)][:, 5 + (ti - 6), :]
                return expw[(i, 3)][:, 7, :]

            def po_key(i, ti):
                return (i, ti // 4), ti % 4, 4

            def emit_pv(i, tis):
                for ti in tis:
                    key, tii, nrows = po_key(i, ti)
                    if key not in po:
                        po[key] = ps_sm.tile(
                            [P, nrows, HS + 1], f32, tag="sm",
                            name=f"po{key[0]}k{key[1]}"
                        )
                    p = po[key]
                    for si in range(ti + 1):
                        nc.tensor.matmul(
                            p[:, tii, :],
                            ew_chunk(i, si, ti),
                            vaug[i][:, si, :],
                            start=(si == 0),
                            stop=(si == ti),
                        )

            def emit_norm_from(key, nrows, dst, store_eng):
                p = po[key]
                o = outp.tile([P, nrows, HS], bf16, tag="osb",
                              name=f"o{key[0]}k{key[1]}")
                r = outp.tile([P, nrows], f32, tag="recip",
                              name=f"r{key[0]}k{key[1]}")
                nc.vector.reciprocal(r, p[:, :, HS])
                nc.vector.tensor_tensor(
                    o, p[:, :, 0:HS],
                    r[:, :, None].to_broadcast([P, nrows, HS]),
                    mybir.AluOpType.mult,
                )
                store_eng.dma_start(dst, o)

            def emit_norm_out(i, half):
                if i == BPC - 1 and half == 1:
                    # last store is the tail: split across two queues
                    # (scalar engine is idle once the exps are done)
                    p = po[(i, 1)]
                    o = outp.tile([P, 4, HS], bf16, tag="osb", name="o3h1")
                    r = outp.tile([P, 4], f32, tag="recip", name="r3h1")
                    nc.vector.reciprocal(r, p[:, :, HS])
                    nc.vector.tensor_tensor(
                        o, p[:, :, 0:HS],
                        r[:, :, None].to_broadcast([P, 4, HS]),
                        mybir.AluOpType.mult,
                    )
                    nc.sync.dma_start(out[i, 1, :, 0:2, :], o[:, 0:2, :])
                    nc.scalar.dma_start(out[i, 1, :, 2:4, :], o[:, 2:4, :])
                else:
                    emit_norm_from((i, half), 4, out[i, half], nc.sync)

            # ---- software-pipelined emission ----------------------------
            # PE order is chosen so the scalar engine (exp; the pacing
            # engine) never waits: score groups are back-to-back, the next
            # item's first two score groups are emitted right after this
            # item's last, and PV / V-projection fill the PE slack.
            # item 0: per-half projection PSUM tiles so each cast
            # depends only on its half's matmuls (Tile tracks whole-tile
            # deps for the counting semaphores), interleaved A/B so the
            # x-piece arrival stalls are absorbed by useful work.
            A_sb[0] = abp.tile([P, T], bf16, tag="A", name="A0")
            B_sb[0] = abp.tile([P, T], bf16, tag="B", name="B0")
            psAa = ps_big.tile([P, TCH], f32, tag="ps", name="pA0h0")
            for co in range(CO):
                nc.tensor.matmul(
                    psAa, wA_co(co), xT_tiles[0][:, co, 0:TCH],
                    start=(co == 0), stop=(co == CO - 1),
                )
            psBa = ps_big.tile([P, TCH], f32, tag="ps", name="pB0h0")
            for co in range(CO):
                nc.tensor.matmul(
                    psBa, wB_co(co), xT_tiles[0][:, co, 0:TCH],
                    start=(co == 0), stop=(co == CO - 1),
                )
            nc.vector.tensor_copy(A_sb[0][:, 0:TCH], psAa)
            load_gated(2, A_sb[0][:, 0:2])
            # si0/si1 columns on the idle scalar engine, rest on DVE
            nc.scalar.copy(B_sb[0][:, 0:2 * P], psBa[:, 0:2 * P])
            nc.vector.tensor_copy(B_sb[0][:, 2 * P:TCH], psBa[:, 2 * P:TCH])
            # si0 scores for the first t-half + their exp, ASAP: the
            # scalar engine starts its exp stream ~4us before the full
            # A/B casts complete
            ew00 = expwp.tile([P, T], bf16, tag="ew0", name="ew0g0")
            expw[(0, 0)] = ew00
            pg0a = ps_big.tile([P, TCH], f32, tag="ps", name="sc0g0a")
            nc.tensor.matmul(
                pg0a, B_sb[0][0:HS, 0:P], A_sb[0][0:HS, 0:TCH],
                start=True, stop=True,
            )
            nc.scalar.activation(ew00[:, 0:TCH], pg0a, Exp, scale=0.125)
            psAb = ps_big.tile([P, TCH], f32, tag="ps", name="pA0h1")
            for co in range(CO):
                nc.tensor.matmul(
                    psAb, wA_co(co), xT_tiles[0][:, co, TCH:T],
                    start=(co == 0), stop=(co == CO - 1),
                )
            psBb = ps_big.tile([P, TCH], f32, tag="ps", name="pB0h1")
            for co in range(CO):
                nc.tensor.matmul(
                    psBb, wB_co(co), xT_tiles[0][:, co, TCH:T],
                    start=(co == 0), stop=(co == CO - 1),
                )
            nc.vector.tensor_copy(A_sb[0][:, TCH:T], psAb)
            nc.vector.tensor_copy(B_sb[0][:, TCH:T], psBb)
            pg0b = ps_big.tile([P, TCH], f32, tag="ps", name="sc0g0b")
            nc.tensor.matmul(
                pg0b, B_sb[0][0:HS, 0:P], A_sb[0][0:HS, TCH:T],
                start=True, stop=True,
            )
            nc.scalar.activation(ew00[:, TCH:T], pg0b, Exp, scale=0.125)
            emit_diag(0, 0, ew00[:, 0:P])
            emit_scores(0, 1)
            emit_scores(0, 2)
            emit_projA(1)
            emit_scores(0, 3)
            emit_projB(1)
            emit_scores(0, 4)
            for i in range(BPC):
                # entering here: scores(i, *) all emitted; A/B(i+1)
                # emitted; PV(i), V(i), norms(i), scores(i+1, *),
                # A/B(i+2) still to do.
                nxt = i + 1 < BPC
                if nxt:
                    emit_scores(i + 1, 0)
                    emit_scores(i + 1, 1)
                if i + 2 < BPC:
                    emit_projA(i + 2)
                emit_projV(i)
                emit_pv(i, [0, 1, 2, 3])
                emit_norm_out(i, 0)
                emit_pv(i, [4, 5])
                if nxt:
                    emit_scores(i + 1, 2)
                    emit_scores(i + 1, 3)
                if i + 2 < BPC:
                    emit_projB(i + 2)
                emit_pv(i, [6, 7])
                emit_norm_out(i, 1)
                if nxt:
                    emit_scores(i + 1, 4)

    nc.compile()
    return nc


def _get_nc():
    nc = _cached.get("nc")
    if nc is None:
        nc = _build()
        _cached["nc"] = nc
    return nc


def _in_maps(x, Wk, Wq, Wv):
    bf = ml_dtypes.bfloat16
    x = np.asarray(x, dtype=np.float32)
    Wk = np.asarray(Wk, dtype=np.float32)
    Wq = np.asarray(Wq, dtype=np.float32)
    Wv = np.asarray(Wv, dtype=np.float32)
    # packed per-partition weight blob [P, 768]:
    # [wA(co0)|wA(co1)|wB(co0)|wB(co1)|wV(co0)|wV(co1)|mask]
    # where wX(co) is W{X}.T[co*128:(co+1)*128, :] laid out so partition p
    # holds contraction row p of chunk co.
    wA = np.concatenate([Wq.T, Wk.T], axis=1)   # [C, 2HS]
    wB = np.concatenate([Wk.T, Wq.T], axis=1)   # [C, 2HS]
    wV = Wv.T                                   # [C, HS]
    m = np.triu(np.ones((P, P), dtype=np.float32))
    wcat = np.concatenate(
        [
            wA[0:P, :], wA[P:C, :],
            wB[0:P, :], wB[P:C, :],
            wV[0:P, :], wV[P:C, :],
            m,
        ],
        axis=1,
    )
    wcat = np.ascontiguousarray(wcat).astype(bf)
    maps = []
    for c in range(NCORES):
        xs = x[c * BPC:(c + 1) * BPC]
        xsT = np.ascontiguousarray(xs.transpose(0, 2, 1)).astype(bf)
        maps.append({"xT": xsT, "wcat": wcat})
    return maps


def _run(x, Wk, Wq, Wv, **spmd_kwargs):
    from concourse.bass_utils import run_bass_kernel_spmd

    nc = _get_nc()
    res = run_bass_kernel_spmd(
        nc, _in_maps(x, Wk, Wq, Wv), core_ids=list(range(NCORES)), **spmd_kwargs
    )
    # out is [BPC, 2, P, 4, HS] bf16 with t = (half*4 + tii)*128 + p
    full = np.concatenate(
        [
            r["out"].transpose(0, 1, 3, 2, 4).reshape(BPC, T, HS)
            for r in res.results
        ],
        axis=0,
    ).astype(np.float32)
    return full, res


def kernel(x, Wk, Wq, Wv):
    full, _ = _run(x, Wk, Wq, Wv)
    return full



# revision 63
# speedup vs baseline: 1.0158x; 1.0158x over previous
"""Causal single-head attention on 8 Trainium2 NeuronCores (batch-parallel), v2.

Problem (nn_Head): x[32,1024,256] f32, Wk/Wq/Wv[64,256] f32.
  q/k/v = x @ W.T ; wei = softmax(causal(q @ k.T / 8)) ; out = wei @ v.

Sharding: B=32 split 4-per-core across 8 cores; weights replicated.

v3 changes vs v2:
  - outputs stored as bf16 in [BPC, 2, P, 4, HS] layout (512B contiguous
    per partition per descriptor, no small-element DMA penalty), upcast
    to f32 on the host: halves the store traffic and shortens the tail.
  - item 0's si0 scores + exp run per t-half with their own PSUM tiles,
    so the scalar engine's exp stream starts as soon as the first x half
    is projected (~2us earlier than waiting for the full A/B casts).
  - diagonal mask-multiplies write SEPARATE ewd tiles instead of masking
    ew in place: Tile tracks whole-tile deps, so in-place masking made
    every PV matmul of a group wait for the pool multiply (~0.7us after
    each exp); with ewd only the one diagonal PV matmul waits.
  - item 0's x pieces ride the sync + scalar queues only (no SWDGE
    desc-gen on the pool engine).
"""

import numpy as np
import ml_dtypes

B, T, C, HS = 32, 1024, 256, 64
NCORES = 8
BPC = B // NCORES  # batch items per core
P = 128            # partitions / row-tile
NT = T // P        # 8 row tiles per item
CO = C // P        # 2 contraction chunks for projections
TCH = 512          # matmul free-dim chunk (one PSUM bank of f32)
N_WARMUP = 7       # dummy matmuls (N=512) to warm the PE clock

_cached = {}


def _build():
    import concourse.tile as tile
    from concourse import bacc, mybir

    bf16 = mybir.dt.bfloat16
    f32 = mybir.dt.float32
    Exp = mybir.ActivationFunctionType.Exp
    Mult = mybir.AluOpType.mult

    nc = bacc.Bacc(
        "TRN2",
        target_bir_lowering=False,
        debug=False,
        num_devices=NCORES,
    )

    xT = nc.dram_tensor("xT", [BPC, C, T], bf16, kind="ExternalInput")
    # packed weights, one DMA: per partition p the 768 bf16 columns are
    # [wA(co0)|wA(co1)|wB(co0)|wB(co1)|wV(co0)|wV(co1)|mask]
    wcat = nc.dram_tensor("wcat", [P, 768], bf16, kind="ExternalInput")
    out = nc.dram_tensor("out", [BPC, 2, P, 4, HS], bf16, kind="ExternalOutput")

    # score-group layout: group id -> (si list, psum cols, act window)
    #   groups 0..3 hold a single si in a [P, 1024] tile (2 banks), valid
    #   t from 128*si, exp reads exactly the causal span.
    #   group 4 = {si4, si5} in [P, 1024]: si4 at cols 0:512 (t=512..1024),
    #   si5 at cols 512:1024 (same t window; valid from t=640) -> one exp
    #   of 1024 cols (128 garbage, never read downstream).
    #   group 5 = {si6, si7} in [P, 512] (1 bank): si6 at 0:256
    #   (t=768..1024), si7 at 256:384 (t=896..1024) -> one exp of 384 cols.

    with tile.TileContext(nc) as tc:
        with (
            tc.tile_pool(name="consts", bufs=1) as consts,
            tc.tile_pool(name="xin", bufs=4) as xin,
            tc.tile_pool(name="ab", bufs=4) as abp,
            tc.tile_pool(name="vau", bufs=3) as vaup,
            tc.tile_pool(name="expw", bufs=2) as expwp,
            tc.tile_pool(name="ewd", bufs=2) as ewdp,
            tc.tile_pool(name="outp", bufs=3) as outp,
            tc.tile_pool(name="ps_big", bufs=3, space="PSUM") as ps_big,
            tc.tile_pool(name="ps_sm", bufs=2, space="PSUM") as ps_sm,
        ):
            # ---- input DMAs ---------------------------------------------
            # packed weights blob: one trigger, first thing on the sync
            # queue so it lands before the (larger) x pieces behind it.
            wcat_sb = consts.tile([P, 768], bf16, tag="wcat")
            nc.sync.dma_start(wcat_sb, wcat[:, :])

            def wA_co(co):
                return wcat_sb[:, co * P:(co + 1) * P]

            def wB_co(co):
                return wcat_sb[:, 256 + co * P:256 + (co + 1) * P]

            def wV_co(co):
                return wcat_sb[:, 512 + co * HS:512 + (co + 1) * HS]

            mask_sb = consts.tile([P, P], bf16, tag="mask")
            nc.gpsimd.tensor_copy(mask_sb, wcat_sb[:, 640:768])

            # warmup source first so the gpsimd memset isn't queued behind
            # the DMA trigger instructions
            dummy_src = consts.tile([P, TCH], bf16, tag="dummy")
            nc.gpsimd.memset(dummy_src, 0.0)

            # x loads: co0 on the sync HW queue, co1 on the scalar HW
            # queue.  The DMA engines round-robin across every queued
            # transfer, so item 0 (t-halved, 4 pieces) goes up alone;
            # each later item's DMA is WAW-gated on a tiny copy that
            # depends on the PREVIOUS item's A-cast, which serializes
            # the items on the wire in the order compute needs them.
            xT_tiles = []
            for it in range(BPC):
                t = xin.tile([P, CO, T], bf16, tag="xT", name=f"xT{it}")
                xT_tiles.append(t)
            for h in range(2):
                nc.sync.dma_start(
                    xT_tiles[0][:, 0:1, h * TCH:(h + 1) * TCH],
                    xT[0].rearrange("(co p) t -> p co t", p=P)[
                        :, 0:1, h * TCH:(h + 1) * TCH],
                )
                # both co1 halves on the scalar queue: skipping the
                # gpsimd SWDGE queue avoids its pool-engine desc-gen cost
                nc.scalar.dma_start(
                    xT_tiles[0][:, 1:2, h * TCH:(h + 1) * TCH],
                    xT[0].rearrange("(co p) t -> p co t", p=P)[
                        :, 1:2, h * TCH:(h + 1) * TCH],
                )

            def load_gated(it, gate_ap, co1_eng=None):
                # items 2/3 go sync-only: a late-gated trigger in the
                # scalar engine's stream would block the exp pipeline
                # (in-order engine).  item 1's gate fires before the
                # first exp, so its co1 half may use the scalar queue.
                t = xT_tiles[it]
                r = xT[it].rearrange("(co p) t -> p co t", p=P)
                nc.gpsimd.tensor_copy(t[:, 0, 0:2], gate_ap)
                nc.gpsimd.tensor_copy(t[:, 1, 0:2], gate_ap)
                nc.sync.dma_start(t[:, 0:1, :], r[:, 0:1, :])
                (co1_eng or nc.sync).dma_start(t[:, 1:2, :], r[:, 1:2, :])

            # item 1: gated on item 0's own co1 data having landed.
            # co1 stays on sync too: a gated trigger at the top of the
            # scalar stream would delay the ACT table load behind it.
            load_gated(1, xT_tiles[0][:, 1, T - 2:T])

            # ---- PE warmup ----------------------------------------------
            ps_warm = ps_big.tile([P, 2, TCH], f32, tag="ps", name="warm")
            for w in range(N_WARMUP):
                nc.tensor.matmul(
                    ps_warm[:, w % 2, :],
                    dummy_src[:, 0:P],
                    dummy_src,
                    start=True,
                    stop=True,
                )

            # ---- per-item emitters --------------------------------------
            A_sb = {}
            B_sb = {}
            vaug = {}
            expw = {}   # (item, grp) -> ew tile
            ewd = {}    # (item, si) -> masked diagonal chunk tile
            po = {}     # (item, key) -> psum tile

            def emit_projA(i, ps=None):
                if ps is None:
                    ps = ps_big.tile([P, 2 * TCH], f32, tag="ps",
                                     name=f"pA{i}")
                for h in range(2):
                    for co in range(CO):
                        nc.tensor.matmul(
                            ps[:, h * TCH:(h + 1) * TCH],
                            wA_co(co),
                            xT_tiles[i][:, co, h * TCH:(h + 1) * TCH],
                            start=(co == 0),
                            stop=(co == CO - 1),
                        )
                A_sb[i] = abp.tile([P, T], bf16, tag="A", name=f"A{i}")
                # split cast: first half unblocks the si=0 score matmul
                nc.vector.tensor_copy(A_sb[i][:, 0:TCH], ps[:, 0:TCH])
                if i + 2 < BPC:
                    load_gated(i + 2, A_sb[i][:, 0:2])
                nc.vector.tensor_copy(A_sb[i][:, TCH:T], ps[:, TCH:T])

            def emit_projB(i, ps=None):
                if ps is None:
                    ps = ps_big.tile([P, 2 * TCH], f32, tag="ps",
                                     name=f"pB{i}")
                for h in range(2):
                    for co in range(CO):
                        nc.tensor.matmul(
                            ps[:, h * TCH:(h + 1) * TCH],
                            wB_co(co),
                            xT_tiles[i][:, co, h * TCH:(h + 1) * TCH],
                            start=(co == 0),
                            stop=(co == CO - 1),
                        )
                B_sb[i] = abp.tile([P, T], bf16, tag="B", name=f"B{i}")
                # split cast: the first 256 cols cover si=0/1; for item 0
                # they go on the still-idle scalar engine so the first
                # scores aren't serialized behind the DVE casts
                if i == 0:
                    nc.scalar.copy(B_sb[i][:, 0:P], ps[:, 0:P])
                    nc.scalar.copy(B_sb[i][:, P:2 * P], ps[:, P:2 * P])
                    nc.vector.tensor_copy(B_sb[i][:, 2 * P:T], ps[:, 2 * P:T])
                else:
                    nc.vector.tensor_copy(B_sb[i][:, 0:2 * P], ps[:, 0:2 * P])
                    nc.vector.tensor_copy(B_sb[i][:, 2 * P:T], ps[:, 2 * P:T])

            def emit_projV(i):
                psv = ps_sm.tile([P, NT, HS], f32, tag="sm", name=f"pV{i}")
                for ti in range(NT):
                    for co in range(CO):
                        nc.tensor.matmul(
                            psv[:, ti, :],
                            xT_tiles[i][:, co, ti * P:(ti + 1) * P],
                            wV_co(co),
                            start=(co == 0),
                            stop=(co == CO - 1),
                        )
                v = vaup.tile([P, NT, HS + 1], bf16, tag="vaug", name=f"va{i}")
                nc.gpsimd.memset(v[:, :, HS:HS + 1], 1.0)
                nc.vector.tensor_copy(v[:, :, 0:HS], psv)
                vaug[i] = v

            def emit_diag(i, si, src_ap):
                """Masked diagonal chunk into its own tile: keeps the pool
                multiply OFF the ew tile so PV's off-diagonal reads don't
                wait on it (Tile tracks whole-tile deps)."""
                d = ewdp.tile([P, P], bf16, tag=f"d{si}", name=f"ewd{i}s{si}")
                ewd[(i, si)] = d
                nc.gpsimd.tensor_mul(d, src_ap, mask_sb)

            def emit_scores(i, grp):
                """matmul + exp + (pool) diagonal mask for one score group."""
                if grp < 3:
                    si = grp
                    t_lo = si * P
                    ncols = T - t_lo
                    ps = ps_big.tile([P, 2 * TCH], f32, tag="ps",
                                     name=f"sc{i}g{grp}")
                    for tj in range(2):
                        t0 = max(tj * TCH, t_lo)
                        t1 = (tj + 1) * TCH
                        if t0 >= t1:
                            continue
                        nc.tensor.matmul(
                            ps[:, t0:t1],
                            B_sb[i][0:HS, si * P:(si + 1) * P],
                            A_sb[i][0:HS, t0:t1],
                            start=True,
                            stop=True,
                        )
                    ew = expwp.tile([P, ncols], bf16, tag=f"ew{grp}",
                                    name=f"ew{i}g{grp}")
                    nc.scalar.activation(ew, ps[:, t_lo:T], Exp, scale=0.125)
                    emit_diag(i, si, ew[:, 0:P])
                elif grp == 3:
                    # merged {si3|si6|si7}: blocks 0:5 = si3 (t=384+128b),
                    # 5:7 = si6 (t=768..1024), 7 = si7 (t=896..1024)
                    ps = ps_big.tile([P, 8, P], f32, tag="ps",
                                     name=f"sc{i}g3")
                    nc.tensor.matmul(
                        ps[:, 0:1, :],
                        B_sb[i][0:HS, 3 * P:4 * P],
                        A_sb[i][0:HS, 3 * P:TCH],
                        start=True, stop=True,
                    )
                    nc.tensor.matmul(
                        ps[:, 1:4, :],
                        B_sb[i][0:HS, 3 * P:4 * P],
                        A_sb[i][0:HS, TCH:TCH + 384],
                        start=True, stop=True,
                    )
                    nc.tensor.matmul(
                        ps[:, 4:5, :],
                        B_sb[i][0:HS, 3 * P:4 * P],
                        A_sb[i][0:HS, TCH + 384:T],
                        start=True, stop=True,
                    )
                    nc.tensor.matmul(
                        ps[:, 5:7, :],
                        B_sb[i][0:HS, 6 * P:7 * P],
                        A_sb[i][0:HS, 6 * P:T],
                        start=True, stop=True,
                    )
                    nc.tensor.matmul(
                        ps[:, 7:8, :],
                        B_sb[i][0:HS, 7 * P:T],
                        A_sb[i][0:HS, 7 * P:T],
                        start=True, stop=True,
                    )
                    ew = expwp.tile([P, 8, P], bf16, tag="ew3",
                                    name=f"ew{i}g3")
                    nc.scalar.activation(ew, ps, Exp, scale=0.125)
                    # diagonals: si3 at block 0, si6 at block 5, si7 at 7
                    emit_diag(i, 3, ew[:, 0, :])
                    emit_diag(i, 6, ew[:, 5, :])
                    emit_diag(i, 7, ew[:, 7, :])
                elif grp == 4:
                    # blocks: si4 at 0:4 (t = 512 + 128b), si5 at 4:7
                    # (t = 640 + 128(b-4)); no garbage block.
                    ps = ps_big.tile([P, 7, P], f32, tag="ps",
                                     name=f"sc{i}g4")
                    nc.tensor.matmul(
                        ps[:, 0:4, :],
                        B_sb[i][0:HS, 4 * P:5 * P],
                        A_sb[i][0:HS, TCH:T],
                        start=True,
                        stop=True,
                    )
                    nc.tensor.matmul(
                        ps[:, 4:7, :],
                        B_sb[i][0:HS, 5 * P:6 * P],
                        A_sb[i][0:HS, 5 * P:T],
                        start=True,
                        stop=True,
                    )
                    ew = expwp.tile([P, 7, P], bf16, tag="ew4",
                                    name=f"ew{i}g4")
                    nc.scalar.activation(ew, ps, Exp, scale=0.125)
                    # diagonals: si4 at block 0, si5 at block 4
                    emit_diag(i, 4, ew[:, 0, :])
                    emit_diag(i, 5, ew[:, 4, :])
                expw[(i, grp)] = ew

            def ew_chunk(i, si, ti):
                """128-wide ew column chunk for (si, ti)."""
                if si == ti:
                    return ewd[(i, si)]
                if si < 3:
                    ew = expw[(i, si)]
                    c0 = (ti - si) * P
                    return ew[:, c0:c0 + P]
                if si == 3:
                    return expw[(i, 3)][:, ti - 3, :]
                if si == 4:
                    return expw[(i, 4)][:, ti - 4, :]
                if si == 5:
                    return expw[(i, 4)][:, ti - 1, :]
                if si == 6:
                    return expw[(i, 3)][:, 5 + (ti - 6), :]
                return expw[(i, 3)][:, 7, :]

            def po_key(i, ti):
                return (i, ti // 4), ti % 4, 4

            def emit_pv(i, tis):
                for ti in tis:
                    key, tii, nrows = po_key(i, ti)
                    if key not in po:
                        po[key] = ps_sm.tile(
                            [P, nrows, HS + 1], f32, tag="sm",
                            name=f"po{key[0]}k{key[1]}"
                        )
                    p = po[key]
                    for si in range(ti + 1):
                        nc.tensor.matmul(
                            p[:, tii, :],
                            ew_chunk(i, si, ti),
                            vaug[i][:, si, :],
                            start=(si == 0),
                            stop=(si == ti),
                        )

            def emit_norm_from(key, nrows, dst, store_eng):
                p = po[key]
                o = outp.tile([P, nrows, HS], bf16, tag="osb",
                              name=f"o{key[0]}k{key[1]}")
                r = outp.tile([P, nrows], f32, tag="recip",
                              name=f"r{key[0]}k{key[1]}")
                nc.vector.reciprocal(r, p[:, :, HS])
                nc.vector.tensor_tensor(
                    o, p[:, :, 0:HS],
                    r[:, :, None].to_broadcast([P, nrows, HS]),
                    mybir.AluOpType.mult,
                )
                store_eng.dma_start(dst, o)

            def emit_norm_out(i, half):
                if i == BPC - 1 and half == 1:
                    # last store is the tail: split across two queues
                    # (scalar engine is idle once the exps are done)
                    p = po[(i, 1)]
                    o = outp.tile([P, 4, HS], bf16, tag="osb", name="o3h1")
                    r = outp.tile([P, 4], f32, tag="recip", name="r3h1")
                    nc.vector.reciprocal(r, p[:, :, HS])
                    nc.vector.tensor_tensor(
                        o, p[:, :, 0:HS],
                        r[:, :, None].to_broadcast([P, 4, HS]),
                        mybir.AluOpType.mult,
                    )
                    nc.sync.dma_start(out[i, 1, :, 0:2, :], o[:, 0:2, :])
                    nc.scalar.dma_start(out[i, 1, :, 2:4, :], o[:, 2:4, :])
                else:
                    emit_norm_from((i, half), 4, out[i, half], nc.sync)

            # ---- software-pipelined emission ----------------------------
            # PE order is chosen so the scalar engine (exp; the pacing
            # engine) never waits: score groups are back-to-back, the next
            # item's first two score groups are emitted right after this
            # item's last, and PV / V-projection fill the PE slack.
            # item 0: per-half projection PSUM tiles so each cast
            # depends only on its half's matmuls (Tile tracks whole-tile
            # deps for the counting semaphores), interleaved A/B so the
            # x-piece arrival stalls are absorbed by useful work.
            A_sb[0] = abp.tile([P, T], bf16, tag="A", name="A0")
            B_sb[0] = abp.tile([P, T], bf16, tag="B", name="B0")
            psAa = ps_big.tile([P, TCH], f32, tag="ps", name="pA0h0")
            for co in range(CO):
                nc.tensor.matmul(
                    psAa, wA_co(co), xT_tiles[0][:, co, 0:TCH],
                    start=(co == 0), stop=(co == CO - 1),
                )
            psBa = ps_big.tile([P, TCH], f32, tag="ps", name="pB0h0")
            for co in range(CO):
                nc.tensor.matmul(
                    psBa, wB_co(co), xT_tiles[0][:, co, 0:TCH],
                    start=(co == 0), stop=(co == CO - 1),
                )
            nc.vector.tensor_copy(A_sb[0][:, 0:TCH], psAa)
            load_gated(2, A_sb[0][:, 0:2])
            # si0/si1 columns on the idle scalar engine, rest on DVE
            nc.scalar.copy(B_sb[0][:, 0:2 * P], psBa[:, 0:2 * P])
            nc.vector.tensor_copy(B_sb[0][:, 2 * P:TCH], psBa[:, 2 * P:TCH])
            # si0 scores for the first t-half + their exp, ASAP: the
            # scalar engine starts its exp stream ~4us before the full
            # A/B casts complete
            ew00 = expwp.tile([P, T], bf16, tag="ew0", name="ew0g0")
            expw[(0, 0)] = ew00
            pg0a = ps_big.tile([P, TCH], f32, tag="ps", name="sc0g0a")
            nc.tensor.matmul(
                pg0a, B_sb[0][0:HS, 0:P], A_sb[0][0:HS, 0:TCH],
                start=True, stop=True,
            )
            nc.scalar.activation(ew00[:, 0:TCH], pg0a, Exp, scale=0.125)
            psAb = ps_big.tile([P, TCH], f32, tag="ps", name="pA0h1")
            for co in range(CO):
                nc.tensor.matmul(
                    psAb, wA_co(co), xT_tiles[0][:, co, TCH:T],
                    start=(co == 0), stop=(co == CO - 1),
                )
            psBb = ps_big.tile([P, TCH], f32, tag="ps", name="pB0h1")
            for co in range(CO):
                nc.tensor.matmul(
                    psBb, wB_co(co), xT_tiles[0][:, co, TCH:T],
                    start=(co == 0), stop=(co == CO - 1),
                )
            nc.vector.tensor_copy(A_sb[0][:, TCH:T], psAb)
            nc.vector.tensor_copy(B_sb[0][:, TCH:T], psBb)
            pg0b = ps_big.tile([P, TCH], f32, tag="ps", name="sc0g0b")
            nc.tensor.matmul(
                pg0b, B_sb[0][0:HS, 0:P], A_sb[0][0:HS, TCH:T],
                start=True, stop=True,
            )
            nc.scalar.activation(ew00[:, TCH:T], pg0b, Exp, scale=0.125)
            emit_diag(0, 0, ew00[:, 0:P])
            emit_scores(0, 1)
            emit_scores(0, 2)
            emit_projA(1)
            emit_scores(0, 3)
            emit_projB(1)
            emit_scores(0, 4)
            for i in range(BPC):
                # entering here: scores(i, *) all emitted; A/B(i+1)
                # emitted; PV(i), V(i), norms(i), scores(i+1, *),
                # A/B(i+2) still to do.
                nxt = i + 1 < BPC
                if nxt:
                    emit_scores(i + 1, 0)
                    emit_scores(i + 1, 1)
                if i + 2 < BPC:
                    emit_projA(i + 2)
                emit_projV(i)
                emit_pv(i, [0, 1, 2, 3])
                emit_norm_out(i, 0)
                emit_pv(i, [4, 5])
                if nxt:
                    emit_scores(i + 1, 2)
                    emit_scores(i + 1, 3)
                if i + 2 < BPC:
                    emit_projB(i + 2)
                emit_pv(i, [6, 7])
                emit_norm_out(i, 1)
                if nxt:
                    emit_scores(i + 1, 4)

    nc.compile()
    return nc


def _get_nc():
    nc = _cached.get("nc")
    if nc is None:
        nc = _build()
        _cached["nc"] = nc
    return nc


def _in_maps(x, Wk, Wq, Wv):
    bf = ml_dtypes.bfloat16
    x = np.asarray(x, dtype=np.float32)
    Wk = np.asarray(Wk, dtype=np.float32)
    Wq = np.asarray(Wq, dtype=np.float32)
    Wv = np.asarray(Wv, dtype=np.float32)
    # packed per-partition weight blob [P, 768]:
    # [wA(co0)|wA(co1)|wB(co0)|wB(co1)|wV(co0)|wV(co1)|mask]
    # where wX(co) is W{X}.T[co*128:(co+1)*128, :] laid out so partition p
    # holds contraction row p of chunk co.
    wA = np.concatenate([Wq.T, Wk.T], axis=1)   # [C, 2HS]
    wB = np.concatenate([Wk.T, Wq.T], axis=1)   # [C, 2HS]
    wV = Wv.T                                   # [C, HS]
    m = np.triu(np.ones((P, P), dtype=np.float32))
    wcat = np.concatenate(
        [
            wA[0:P, :], wA[P:C, :],
            wB[0:P, :], wB[P:C, :],
            wV[0:P, :], wV[P:C, :],
            m,
        ],
        axis=1,
    )
    wcat = np.ascontiguousarray(wcat).astype(bf)
    maps = []
    for c in range(NCORES):
        xs = x[c * BPC:(c + 1) * BPC]
        xsT = np.ascontiguousarray(xs.transpose(0, 2, 1)).astype(bf)
        maps.append({"xT": xsT, "wcat": wcat})
    return maps


def _run(x, Wk, Wq, Wv, **spmd_kwargs):
    from concourse.bass_utils import run_bass_kernel_spmd

    nc = _get_nc()
    res = run_bass_kernel_spmd(
        nc, _in_maps(x, Wk, Wq, Wv), core_ids=list(range(NCORES)), **spmd_kwargs
    )
    # out is [BPC, 2, P, 4, HS] bf16 with t = (half*4 + tii)*128 + p
    full = np.concatenate(
        [
            r["out"].transpose(0, 1, 3, 2, 4).reshape(BPC, T, HS)
            for r in res.results
        ],
        axis=0,
    ).astype(np.float32)
    return full, res


def kernel(x, Wk, Wq, Wv):
    full, _ = _run(x, Wk, Wq, Wv)
    return full

